# revision 15
# baseline (speedup 1.0000x reference)
"""3-layer GCN (PyG GCNConv-style) on 8 Trainium2 NeuronCores.

Distribution: 1-D node partition (2048 nodes per core). Per core:
  - GEMM1: h1T[36,2048] = W1s^T @ x[rows_c]^T in plain fp8 e4m3 with
    column-tiled CONCURRENT matmuls: even K-chunks write PSUM partitions
    [0:36] (PE cols 0-63), odd chunks write [64:100] (cols 64-127) — two
    matmuls stream simultaneously (separate XBUSes), ~2x PE throughput
    for M=36. The halves are summed on DVE afterwards.
  - Per layer: messages g = (dis/64) * h are computed in fp16 [48, 2048],
    transposed node-major with one dma_start_transpose, AllGathered in
    fp16, cast to fp8 in chunks, then aggregated with a dense per-core
    adjacency block A[16384, 2048] fp8 (exact small edge multiplicities;
    symmetric norm folded into pre/post dis scaling) using the same
    parity-split concurrent matmuls.
  - A is split: 58 chunks stay resident in SBUF across all 3 layers; the
    rest re-stream each layer (streamed first, prefetched under the
    collective). DMA issue is spread across queues: bulk x + staging on
    Sync, resident A on Activation, streamed A on GpSimd.
  - Small GEMMs (W2, W3 pre-scaled, fp16); fp16 logits, one dma
    transpose, f32 softmax tail.

Scaling discipline (host folds all constants):
  W1s=64*W1, W2s=512*W2, W3s=512*W3, disr_pre=dis/64, disr_post=dis,
  b2s=8*b2. Every fp8 tensor has rms ~0.25-1 and
  logits = disr_pre * agg3 + b3 exactly.
"""
import numpy as np
import concourse.bacc as bacc
import concourse.mybir as mybir
import concourse.tile as tile
from concourse.bass_utils import run_bass_kernel_spmd

N = 16384
E = 524288
H = 36
C = 16
NCORES = 8
ND = N // NCORES          # 2048 nodes per core
NCH = 128                 # K-chunks of 128
NP2 = 64                  # chunk-pairs (DMA granularity)
RB = ND // 128            # 16 row-blocks per core
HP = 48                   # H padded (xbar rows, fp8 row stride)
NRESP = 27                # resident A chunk-PAIRS (58 chunks)
NSTREAM = 6               # rotating stream buffers
FP8 = mybir.dt.np(mybir.dt.float8e4)

_PROGRAM = None
_LAST_RES = None


def _build_program(nresp=NRESP):
    nc = bacc.Bacc(None)
    f32, f16, fp8 = mybir.dt.float32, mybir.dt.float16, mybir.dt.float8e4
    nres = 2 * nresp

    xT_d = nc.dram_tensor("xT", [NP2, 128, 2, 4, 512], fp8, kind="ExternalInput")
    W1c_d = nc.dram_tensor("W1c", [128, NCH, HP], fp8, kind="ExternalInput")
    A_d = nc.dram_tensor("A", [NP2, 128, 2, 4, 512], fp8, kind="ExternalInput")
    dpre_d = nc.dram_tensor("dpre", [H, ND], f32, kind="ExternalInput")
    dpost_d = nc.dram_tensor("dpost", [H, ND], f32, kind="ExternalInput")
    W2_d = nc.dram_tensor("W2s", [H, H], f16, kind="ExternalInput")
    W3_d = nc.dram_tensor("W3s", [H, C], f16, kind="ExternalInput")
    b1_d = nc.dram_tensor("b1", [H, 1], f32, kind="ExternalInput")
    b2_d = nc.dram_tensor("b2s", [H, 1], f32, kind="ExternalInput")
    b3_d = nc.dram_tensor("b3f", [C, 1], f32, kind="ExternalInput")
    out_d = nc.dram_tensor("out", [ND, C], f32, kind="ExternalOutput")

    cc_in = [[nc.dram_tensor(f"cc{l}{h}_in", [ND // 2, HP], f16)
              for h in "ab"] for l in range(3)]
    cc_out = [[nc.dram_tensor(f"cc{l}{h}_out", [N // 2, HP], f16,
                              addr_space="Shared") for h in "ab"]
              for l in range(3)]
    groups = [list(range(NCORES))]

    with tile.TileContext(nc) as tc:
        with (
            tc.tile_pool(name="const", bufs=1) as constp,
            tc.tile_pool(name="ares", bufs=1) as aresp,
            tc.tile_pool(name="stream", bufs=NSTREAM) as streamp,
            tc.tile_pool(name="gt", bufs=1) as gtp,
            tc.tile_pool(name="work", bufs=1) as work,
            tc.tile_pool(name="psb", bufs=1, space="PSUM") as psb,
        ):
            W1c = constp.tile([128, NCH, HP], fp8)
            dpre = constp.tile([H, ND], f32)
            dpost = constp.tile([H, ND], f32)
            W2t = constp.tile([H, H], f16)
            W3t = constp.tile([H, C], f16)
            b1t = constp.tile([H, 1], f32)
            b2t = constp.tile([H, 1], f32)
            b3t = constp.tile([C, 1], f32)
            nc.sync.dma_start(W1c[:], W1c_d[:])
            nc.sync.dma_start(dpre[:], dpre_d[:])
            nc.sync.dma_start(dpost[:], dpost_d[:])
            nc.sync.dma_start(W2t[:], W2_d[:])
            nc.sync.dma_start(W3t[:], W3_d[:])
            nc.sync.dma_start(b1t[:], b1_d[:])
            nc.sync.dma_start(b2t[:], b2_d[:])
            nc.sync.dma_start(b3t[:], b3_d[:])

            A_res = aresp.tile([128, nresp, 2, 4, 512], fp8)

            # (resident A loads are emitted right after the gemm1 loop on
            # the same sync DMA ring: ring FIFO serializes them behind x)
            # ---- GEMM1: parity-split concurrent plain-fp8 matmuls ----
            with nc.named_scope("gemm1"):
                hT = psb.tile([128, ND], f32, tag="big")
                for c2 in range(NP2):
                    xt = streamp.tile([128, 2, 4, 512], fp8, tag="st")
                    nc.sync.dma_start(xt[:], xT_d[c2])
                    for j in range(2):
                        ch = 2 * c2 + j
                        base = 64 * (ch % 2)
                        for q in range(4):
                            nc.tensor.matmul(
                                hT[base:base + H, q * 512:(q + 1) * 512],
                                W1c[:, ch, 0:H],
                                xt[:, j, q, :],
                                start=(c2 == 0),
                                stop=(c2 == NP2 - 1),
                            )

            for r in range(nresp):
                nc.sync.dma_start(A_res[:, r, :, :, :], A_d[r])

            for layer in range(3):
                F = H if layer < 2 else C
                # ---- stage: g16 = dpre*h -> dma-transpose -> AllGather ----
                with nc.named_scope(f"stage{layer}"):
                    if layer == 0:
                        bsum = work.tile([H, ND], f32, tag="bsum")
                        nc.scalar.activation(
                            bsum[:], hT[64:64 + H, :],
                            mybir.ActivationFunctionType.Copy,
                        )
                        tsum = work.tile([H, ND], f32, tag="tsum")
                        nc.vector.tensor_tensor(
                            tsum[:], hT[0:H, :], bsum[:],
                            mybir.AluOpType.add,
                        )
                        hsrc = tsum
                    else:
                        hsrc = hT[0:F, :]
                    g16 = work.tile([HP, ND], f16, tag="g16")
                    nc.gpsimd.memset(g16[:], 0)
                    nc.vector.tensor_tensor(
                        g16[0:F, :], hsrc[0:F, :], dpre[0:F, :],
                        mybir.AluOpType.mult,
                    )
                    gown = work.tile([128, RB, HP], f16, tag="gown")
                    nc.scalar.dma_start_transpose(gown[:], g16[:])
                    g16th = [gtp.tile([128, NCH // 2, HP], f16,
                                      tag=f"g16t{h}", name=f"g16t{h}")
                             for h in range(2)]
                    for h in range(2):
                        nc.scalar.dma_start(
                            cc_in[layer][h][:].rearrange(
                                "(b p) f -> p b f", p=64),
                            gown[64 * h:64 * h + 64, :, :],
                        )
                        nc.gpsimd.collective_compute(
                            "AllGather",
                            mybir.AluOpType.bypass,
                            replica_groups=groups,
                            ins=[cc_in[layer][h][:]],
                            outs=[cc_out[layer][h][:]],
                        )
                        nc.scalar.dma_start(
                            g16th[h][:],
                            cc_out[layer][h][:].rearrange(
                                "(p c) f -> p c f", p=128),
                        )

                # ---- aggregation: parity-split concurrent matmuls ----
                with nc.named_scope(f"agg{layer}"):
                    aggT = psb.tile([128, ND], f32, tag="big")
                    c2_order = list(range(NP2))
                    for idx, c2 in enumerate(c2_order):
                        if c2 >= nresp:
                            a_t = streamp.tile([128, 2, 4, 512], fp8, tag="st")
                            nc.sync.dma_start(a_t[:], A_d[c2])
                            rhs = a_t
                        else:
                            rhs = A_res[:, c2, :, :, :]
                        for j in range(2):
                            ch = 2 * c2 + j
                            base = 64 * (ch % 2)
                            for q in range(4):
                                nc.tensor.matmul(
                                    aggT[base:base + F, q * 512:(q + 1) * 512],
                                    g16th[ch // 64][:, ch % 64, 0:F],
                                    rhs[:, j, q, :],
                                    start=(idx == 0),
                                    stop=(idx == NP2 - 1),
                                )

                with nc.named_scope(f"post{layer}"):
                    bsum = work.tile([H, ND], f32, tag="bsum")
                    nc.scalar.activation(
                        bsum[0:F, :], aggT[64:64 + F, :],
                        mybir.ActivationFunctionType.Copy,
                    )
                    tsum = work.tile([H, ND], f32, tag="tsum")
                    nc.vector.tensor_tensor(
                        tsum[0:F, :], aggT[0:F, :], bsum[0:F, :],
                        mybir.AluOpType.add,
                    )
                    if layer < 2:
                        # in_{l+1} = relu(dpost*agg + b); next hT = W^T @ in
                        tmp16 = work.tile([H, ND], f16, tag="tmp16")
                        nc.vector.tensor_tensor(
                            tmp16[:], tsum[0:H, :], dpost[:],
                            mybir.AluOpType.mult,
                        )
                        inx = work.tile([H, ND], f16, tag="inx")
                        nc.scalar.activation(
                            inx[:], tmp16[:], mybir.ActivationFunctionType.Relu,
                            bias=b1t[:] if layer == 0 else b2t[:],
                        )
                        Wt = W2t if layer == 0 else W3t
                        Fn = H if layer == 0 else C
                        hT = psb.tile([128, ND], f32, tag="big")
                        for q in range(4):
                            nc.tensor.matmul(
                                hT[0:Fn, q * 512:(q + 1) * 512],
                                Wt[:, 0:Fn],
                                inx[:, q * 512:(q + 1) * 512],
                                start=True,
                                stop=True,
                            )
                    else:
                        # logits = dpre*agg + b3; dma-transpose; softmax
                        tmpL = work.tile([C, ND], f16, tag="tmpL")
                        nc.vector.tensor_tensor(
                            tmpL[:], tsum[0:C, :], dpre[0:C, :],
                            mybir.AluOpType.mult,
                        )
                        logT = work.tile([C, ND], f16, tag="logT")
                        nc.vector.tensor_scalar(
                            logT[:], tmpL[:], b3t[:], None,
                            mybir.AluOpType.add,
                        )
                        onat = work.tile([128, RB, C], f16, tag="onat")
                        nc.scalar.dma_start_transpose(onat[:], logT[:])
                        negmax = work.tile([128, RB], f16, tag="negmax")
                        nc.vector.tensor_reduce(
                            negmax[:], onat[:], axis=mybir.AxisListType.X,
                            op=mybir.AluOpType.max, negate=True,
                        )
                        expv = work.tile([128, RB, C], f32, tag="expv")
                        ssum = work.tile([128, RB], f32, tag="ssum")
                        for rb in range(RB):
                            nc.scalar.activation(
                                expv[:, rb, :], onat[:, rb, :],
                                mybir.ActivationFunctionType.Exp,
                                bias=negmax[:, rb:rb + 1],
                                accum_out=ssum[:, rb:rb + 1],
                            )
                        rsum = work.tile([128, RB], f32, tag="rsum")
                        nc.vector.reciprocal(rsum[:], ssum[:])
                        prob = work.tile([128, RB, C], f32, tag="prob")
                        for rb in range(RB):
                            nc.vector.tensor_scalar(
                                prob[:, rb, :], expv[:, rb, :],
                                rsum[:, rb:rb + 1], None,
                                mybir.AluOpType.mult,
                            )
                        nc.sync.dma_start(
                            out_d[:].rearrange("(b p) f -> p b f", p=128),
                            prob[:],
                        )

    nc.finalize()
    return nc


def _get_program():
    global _PROGRAM
    if _PROGRAM is None:
        _PROGRAM = _build_program()
    return _PROGRAM


def _pair_q_layout(mat):
    """[16384, 2048] -> [64, 128, 2, 4, 512]: [c2, p, j, q, n] with
    row = p*128 + 2*c2 + j, col = q*512 + n."""
    return np.ascontiguousarray(
        mat.reshape(128, NP2, 2, 4, 512).transpose(1, 0, 2, 3, 4)
    )


def kernel(x, edge_index, W1, b1, W2, b2, W3, b3, _profile=False):
    x = np.asarray(x, dtype=np.float32)
    edge_index = np.asarray(edge_index)
    W1 = np.asarray(W1, dtype=np.float32)
    W2 = np.asarray(W2, dtype=np.float32)
    W3 = np.asarray(W3, dtype=np.float32)
    b1 = np.asarray(b1, dtype=np.float32)
    b2 = np.asarray(b2, dtype=np.float32)
    b3 = np.asarray(b3, dtype=np.float32)

    # ---- graph preprocessing (host) ----
    loop = np.arange(N, dtype=np.int64)
    src = np.concatenate([edge_index[0].astype(np.int64), loop])
    dst = np.concatenate([edge_index[1].astype(np.int64), loop])
    deg = np.bincount(dst, minlength=N).astype(np.float32)
    dis = (1.0 / np.sqrt(np.maximum(deg, np.float32(1.0)))).astype(np.float32)

    order = np.argsort(dst)
    src_s, dst_s = src[order], dst[order]
    core_of = dst_s // ND
    bounds = np.searchsorted(core_of, np.arange(NCORES + 1))

    W1c = np.zeros((128, NCH, HP), dtype=FP8)
    W1c[:, :, 0:H] = (W1 * np.float32(64.0)).astype(FP8).reshape(128, NCH, H)
    W2s = (W2 * np.float32(512.0)).astype(np.float16)
    W3s = (W3 * np.float32(512.0)).astype(np.float16)

    xT32 = np.ascontiguousarray(x.T)  # [k, node] fp32

    in_maps = []
    for c in range(NCORES):
        lo, hi = bounds[c], bounds[c + 1]
        Af = np.zeros((N, ND), dtype=np.float32)
        np.add.at(Af, (src_s[lo:hi], dst_s[lo:hi] - c * ND), np.float32(1.0))
        A8 = _pair_q_layout(Af.astype(FP8))
        xc8 = xT32[:, c * ND:(c + 1) * ND].astype(FP8)  # [16384, 2048]
        xT8 = _pair_q_layout(xc8)
        disc = dis[c * ND:(c + 1) * ND]
        dpre = np.ascontiguousarray(
            np.broadcast_to((disc / np.float32(64.0))[None, :], (H, ND))
        ).astype(np.float32)
        dpost = np.ascontiguousarray(
            np.broadcast_to(disc[None, :], (H, ND))
        ).astype(np.float32)
        in_maps.append({
            "xT": xT8,
            "W1c": W1c,
            "A": A8,
            "dpre": dpre,
            "dpost": dpost,
            "W2s": W2s,
            "W3s": W3s,
            "b1": b1.reshape(H, 1),
            "b2s": (b2 * np.float32(8.0)).reshape(H, 1),
            "b3f": b3.reshape(C, 1),
        })

    nc = _get_program()
    global _LAST_RES
    res = run_bass_kernel_spmd(nc, in_maps, list(range(NCORES)),
                               trace=bool(_profile))
    _LAST_RES = res
    out = np.concatenate([res.results[c]["out"] for c in range(NCORES)], axis=0)
    if _profile:
        return out, res.exec_time_ns
    return out


# revision 17
# speedup vs baseline: 1.0479x; 1.0479x over previous
"""3-layer GCN (PyG GCNConv-style) on 8 Trainium2 NeuronCores.

Distribution: 1-D node partition (2048 nodes per core). Per core:
  - GEMM1: h1T[36,2048] = W1s^T @ x[rows_c]^T in plain fp8 e4m3 with
    column-tiled CONCURRENT matmuls: even K-chunks write PSUM partitions
    [0:36] (PE cols 0-63), odd chunks write [64:100] (cols 64-127) — two
    matmuls stream simultaneously (separate XBUSes), ~2x PE throughput
    for M=36. The halves are summed on DVE afterwards.
  - Per layer: messages g = (dis/64) * h are computed in fp16 [48, 2048],
    transposed node-major with one dma_start_transpose, AllGathered in
    fp16, cast to fp8 in chunks, then aggregated with a dense per-core
    adjacency block A[16384, 2048] fp8 (exact small edge multiplicities;
    symmetric norm folded into pre/post dis scaling) using the same
    parity-split concurrent matmuls.
  - A is split: 58 chunks stay resident in SBUF across all 3 layers; the
    rest re-stream each layer (streamed first, prefetched under the
    collective). DMA issue is spread across queues: bulk x + staging on
    Sync, resident A on Activation, streamed A on GpSimd.
  - Small GEMMs (W2, W3 pre-scaled, fp16); fp16 logits, one dma
    transpose, f32 softmax tail.

Scaling discipline (host folds all constants):
  W1s=64*W1, W2s=512*W2, W3s=512*W3, disr_pre=dis/64, disr_post=dis,
  b2s=8*b2. Every fp8 tensor has rms ~0.25-1 and
  logits = disr_pre * agg3 + b3 exactly.
"""
import numpy as np
import concourse.bacc as bacc
import concourse.mybir as mybir
import concourse.tile as tile
from concourse.bass_utils import run_bass_kernel_spmd

N = 16384
E = 524288
H = 36
C = 16
NCORES = 8
ND = N // NCORES          # 2048 nodes per core
NCH = 128                 # K-chunks of 128
NP2 = 64                  # chunk-pairs (DMA granularity)
RB = ND // 128            # 16 row-blocks per core
HP = 48                   # H padded (xbar rows, fp8 row stride)
NRESP = 31                # resident A chunk-PAIRS (58 chunks)
NSTREAM = 6               # rotating stream buffers
FP8 = mybir.dt.np(mybir.dt.float8e4)

_PROGRAM = None
_LAST_RES = None


def _build_program(nresp=NRESP):
    nc = bacc.Bacc(None)
    f32, f16, fp8 = mybir.dt.float32, mybir.dt.float16, mybir.dt.float8e4
    nres = 2 * nresp

    xT_d = nc.dram_tensor("xT", [NP2, 128, 2, 4, 512], fp8, kind="ExternalInput")
    W1c_d = nc.dram_tensor("W1c", [128, NCH, HP], fp8, kind="ExternalInput")
    A_d = nc.dram_tensor("A", [NP2, 128, 2, 4, 512], fp8, kind="ExternalInput")
    dpre_d = nc.dram_tensor("dpre", [H, ND], f32, kind="ExternalInput")
    W2_d = nc.dram_tensor("W2s", [H, H], f16, kind="ExternalInput")
    W3_d = nc.dram_tensor("W3s", [H, C], f16, kind="ExternalInput")
    b1_d = nc.dram_tensor("b1", [H, 1], f32, kind="ExternalInput")
    b2_d = nc.dram_tensor("b2s", [H, 1], f32, kind="ExternalInput")
    b3_d = nc.dram_tensor("b3f", [C, 1], f32, kind="ExternalInput")
    out_d = nc.dram_tensor("out", [ND, C], f32, kind="ExternalOutput")

    cc_in = [[nc.dram_tensor(f"cc{l}{h}_in", [ND // 2, HP], f16)
              for h in "ab"] for l in range(3)]
    cc_out = [[nc.dram_tensor(f"cc{l}{h}_out", [N // 2, HP], f16,
                              addr_space="Shared") for h in "ab"]
              for l in range(3)]
    groups = [list(range(NCORES))]

    with tile.TileContext(nc) as tc:
        with (
            tc.tile_pool(name="const", bufs=1) as constp,
            tc.tile_pool(name="ares", bufs=1) as aresp,
            tc.tile_pool(name="stream", bufs=NSTREAM) as streamp,
            tc.tile_pool(name="gt", bufs=1) as gtp,
            tc.tile_pool(name="work", bufs=1) as work,
            tc.tile_pool(name="psb", bufs=1, space="PSUM") as psb,
        ):
            W1c = constp.tile([128, NCH, HP], fp8)
            dpre = constp.tile([H, ND], f32)
            W2t = constp.tile([H, H], f16)
            W3t = constp.tile([H, C], f16)
            b1t = constp.tile([H, 1], f32)
            b2t = constp.tile([H, 1], f32)
            b3t = constp.tile([C, 1], f32)
            nc.sync.dma_start(W1c[:], W1c_d[:])
            nc.sync.dma_start(dpre[:], dpre_d[:])
            nc.sync.dma_start(W2t[:], W2_d[:])
            nc.sync.dma_start(W3t[:], W3_d[:])
            nc.sync.dma_start(b1t[:], b1_d[:])
            nc.sync.dma_start(b2t[:], b2_d[:])
            nc.sync.dma_start(b3t[:], b3_d[:])

            A_res = aresp.tile([128, nresp, 2, 4, 512], fp8)

            # (resident A loads are emitted right after the gemm1 loop on
            # the same sync DMA ring: ring FIFO serializes them behind x)
            # ---- GEMM1: parity-split concurrent plain-fp8 matmuls ----
            with nc.named_scope("gemm1"):
                hT = psb.tile([128, ND], f32, tag="big")
                for c2 in range(NP2):
                    xt = streamp.tile([128, 2, 4, 512], fp8, tag="st")
                    nc.sync.dma_start(xt[:], xT_d[c2])
                    for j in range(2):
                        ch = 2 * c2 + j
                        base = 64 * (ch % 2)
                        for q in range(4):
                            nc.tensor.matmul(
                                hT[base:base + H, q * 512:(q + 1) * 512],
                                W1c[:, ch, 0:H],
                                xt[:, j, q, :],
                                start=(c2 == 0),
                                stop=(c2 == NP2 - 1),
                            )

            for r in range(nresp):
                nc.sync.dma_start(A_res[:, r, :, :, :], A_d[r])

            for layer in range(3):
                F = H if layer < 2 else C
                # ---- stage: g16 = dpre*h -> dma-transpose -> AllGather ----
                with nc.named_scope(f"stage{layer}"):
                    if layer == 0:
                        bsum = work.tile([H, ND], f32, tag="bsum")
                        nc.scalar.activation(
                            bsum[:], hT[64:64 + H, :],
                            mybir.ActivationFunctionType.Copy,
                        )
                        tsum = work.tile([H, ND], f32, tag="tsum")
                        nc.vector.tensor_tensor(
                            tsum[:], hT[0:H, :], bsum[:],
                            mybir.AluOpType.add,
                        )
                        hsrc = tsum
                    else:
                        hsrc = hT[0:F, :]
                    g16 = work.tile([HP, ND], f16, tag="g16")
                    nc.vector.tensor_tensor(
                        g16[0:F, :], hsrc[0:F, :], dpre[0:F, :],
                        mybir.AluOpType.mult,
                    )
                    gown = work.tile([128, RB, HP], f16, tag="gown")
                    nc.scalar.dma_start_transpose(gown[:], g16[:])
                    g16th = [gtp.tile([128, NCH // 2, HP], f16,
                                      tag=f"g16t{h}", name=f"g16t{h}")
                             for h in range(2)]
                    for h in range(2):
                        nc.scalar.dma_start(
                            cc_in[layer][h][:].rearrange(
                                "(b p) f -> p b f", p=64),
                            gown[64 * h:64 * h + 64, :, :],
                        )
                        nc.gpsimd.collective_compute(
                            "AllGather",
                            mybir.AluOpType.bypass,
                            replica_groups=groups,
                            ins=[cc_in[layer][h][:]],
                            outs=[cc_out[layer][h][:]],
                        )
                        nc.scalar.dma_start(
                            g16th[h][:],
                            cc_out[layer][h][:].rearrange(
                                "(p c) f -> p c f", p=128),
                        )

                # ---- aggregation: parity-split concurrent matmuls ----
                with nc.named_scope(f"agg{layer}"):
                    aggT = psb.tile([128, ND], f32, tag="big")
                    c2_order = list(range(NP2))
                    for idx, c2 in enumerate(c2_order):
                        if c2 >= nresp:
                            a_t = streamp.tile([128, 2, 4, 512], fp8, tag="st")
                            nc.sync.dma_start(a_t[:], A_d[c2])
                            rhs = a_t
                        else:
                            rhs = A_res[:, c2, :, :, :]
                        for j in range(2):
                            ch = 2 * c2 + j
                            base = 64 * (ch % 2)
                            for q in range(4):
                                nc.tensor.matmul(
                                    aggT[base:base + F, q * 512:(q + 1) * 512],
                                    g16th[ch // 64][:, ch % 64, 0:F],
                                    rhs[:, j, q, :],
                                    start=(idx == 0),
                                    stop=(idx == NP2 - 1),
                                )

                with nc.named_scope(f"post{layer}"):
                    bsum = work.tile([H, ND], f32, tag="bsum")
                    nc.scalar.activation(
                        bsum[0:F, :], aggT[64:64 + F, :],
                        mybir.ActivationFunctionType.Copy,
                    )
                    tsum = work.tile([H, ND], f32, tag="tsum")
                    nc.vector.tensor_tensor(
                        tsum[0:F, :], aggT[0:F, :], bsum[0:F, :],
                        mybir.AluOpType.add,
                    )
                    if layer < 2:
                        # in_{l+1} = relu(dpost*agg + b); next hT = W^T @ in
                        tmp16 = work.tile([H, ND], f16, tag="tmp16")
                        nc.vector.tensor_tensor(
                            tmp16[:], tsum[0:H, :], dpre[0:H, :],
                            mybir.AluOpType.mult,
                        )
                        inx = work.tile([H, ND], f16, tag="inx")
                        nc.scalar.activation(
                            inx[:], tmp16[:], mybir.ActivationFunctionType.Relu,
                            bias=b1t[:] if layer == 0 else b2t[:],
                            scale=64.0,
                        )
                        Wt = W2t if layer == 0 else W3t
                        Fn = H if layer == 0 else C
                        hT = psb.tile([128, ND], f32, tag="big")
                        for q in range(4):
                            nc.tensor.matmul(
                                hT[0:Fn, q * 512:(q + 1) * 512],
                                Wt[:, 0:Fn],
                                inx[:, q * 512:(q + 1) * 512],
                                start=True,
                                stop=True,
                            )
                    else:
                        # logits = dpre*agg + b3; dma-transpose; softmax
                        tmpL = work.tile([C, ND], f16, tag="tmp16")
                        nc.vector.tensor_tensor(
                            tmpL[:], tsum[0:C, :], dpre[0:C, :],
                            mybir.AluOpType.mult,
                        )
                        logT = work.tile([C, ND], f16, tag="inx")
                        nc.vector.tensor_scalar(
                            logT[:], tmpL[:], b3t[:], None,
                            mybir.AluOpType.add,
                        )
                        onat = work.tile([128, RB, C], f16, tag="onat")
                        nc.scalar.dma_start_transpose(onat[:], logT[:])
                        negmax = work.tile([128, RB], f16, tag="negmax")
                        nc.vector.tensor_reduce(
                            negmax[:], onat[:], axis=mybir.AxisListType.X,
                            op=mybir.AluOpType.max, negate=True,
                        )
                        expv = work.tile([128, RB, C], f32, tag="expv")
                        ssum = work.tile([128, RB], f32, tag="ssum")
                        for rb in range(RB):
                            nc.scalar.activation(
                                expv[:, rb, :], onat[:, rb, :],
                                mybir.ActivationFunctionType.Exp,
                                bias=negmax[:, rb:rb + 1],
                                accum_out=ssum[:, rb:rb + 1],
                            )
                        rsum = work.tile([128, RB], f32, tag="rsum")
                        nc.vector.reciprocal(rsum[:], ssum[:])
                        prob = work.tile([128, RB, C], f32, tag="prob")
                        for rb in range(RB):
                            nc.vector.tensor_scalar(
                                prob[:, rb, :], expv[:, rb, :],
                                rsum[:, rb:rb + 1], None,
                                mybir.AluOpType.mult,
                            )
                        nc.sync.dma_start(
                            out_d[:].rearrange("(b p) f -> p b f", p=128),
                            prob[:],
                        )

    nc.finalize()
    return nc


def _get_program():
    global _PROGRAM
    if _PROGRAM is None:
        _PROGRAM = _build_program()
    return _PROGRAM


def _pair_q_layout(mat):
    """[16384, 2048] -> [64, 128, 2, 4, 512]: [c2, p, j, q, n] with
    row = p*128 + 2*c2 + j, col = q*512 + n."""
    return np.ascontiguousarray(
        mat.reshape(128, NP2, 2, 4, 512).transpose(1, 0, 2, 3, 4)
    )


def kernel(x, edge_index, W1, b1, W2, b2, W3, b3, _profile=False):
    x = np.asarray(x, dtype=np.float32)
    edge_index = np.asarray(edge_index)
    W1 = np.asarray(W1, dtype=np.float32)
    W2 = np.asarray(W2, dtype=np.float32)
    W3 = np.asarray(W3, dtype=np.float32)
    b1 = np.asarray(b1, dtype=np.float32)
    b2 = np.asarray(b2, dtype=np.float32)
    b3 = np.asarray(b3, dtype=np.float32)

    # ---- graph preprocessing (host) ----
    loop = np.arange(N, dtype=np.int64)
    src = np.concatenate([edge_index[0].astype(np.int64), loop])
    dst = np.concatenate([edge_index[1].astype(np.int64), loop])
    deg = np.bincount(dst, minlength=N).astype(np.float32)
    dis = (1.0 / np.sqrt(np.maximum(deg, np.float32(1.0)))).astype(np.float32)

    order = np.argsort(dst)
    src_s, dst_s = src[order], dst[order]
    core_of = dst_s // ND
    bounds = np.searchsorted(core_of, np.arange(NCORES + 1))

    W1c = np.zeros((128, NCH, HP), dtype=FP8)
    W1c[:, :, 0:H] = (W1 * np.float32(64.0)).astype(FP8).reshape(128, NCH, H)
    W2s = (W2 * np.float32(512.0)).astype(np.float16)
    W3s = (W3 * np.float32(512.0)).astype(np.float16)

    xT32 = np.ascontiguousarray(x.T)  # [k, node] fp32

    in_maps = []
    for c in range(NCORES):
        lo, hi = bounds[c], bounds[c + 1]
        Af = np.zeros((N, ND), dtype=np.float32)
        np.add.at(Af, (src_s[lo:hi], dst_s[lo:hi] - c * ND), np.float32(1.0))
        A8 = _pair_q_layout(Af.astype(FP8))
        xc8 = xT32[:, c * ND:(c + 1) * ND].astype(FP8)  # [16384, 2048]
        xT8 = _pair_q_layout(xc8)
        disc = dis[c * ND:(c + 1) * ND]
        dpre = np.ascontiguousarray(
            np.broadcast_to((disc / np.float32(64.0))[None, :], (H, ND))
        ).astype(np.float32)
        in_maps.append({
            "xT": xT8,
            "W1c": W1c,
            "A": A8,
            "dpre": dpre,
            "W2s": W2s,
            "W3s": W3s,
            "b1": b1.reshape(H, 1),
            "b2s": (b2 * np.float32(8.0)).reshape(H, 1),
            "b3f": b3.reshape(C, 1),
        })

    nc = _get_program()
    global _LAST_RES
    res = run_bass_kernel_spmd(nc, in_maps, list(range(NCORES)),
                               trace=bool(_profile))
    _LAST_RES = res
    out = np.concatenate([res.results[c]["out"] for c in range(NCORES)], axis=0)
    if _profile:
        return out, res.exec_time_ns
    return out
